# revision 11
# baseline (speedup 1.0000x reference)
"""DMPNN layer kernel for 8 Trainium2 NeuronCores.

Sharding: data-parallel over destination nodes j (dim 2 of edge_attr/adj,
dim 1 of the output). Each core gets a 64-column j-slice of edge_attr/adj,
the full h (needed because messages sum over all source nodes i), and the
small weights replicated. The batch-global mask (adj.sum(0) > 0) only needs
the core's own j-slice of adj over the full batch, so no collective at all.

Math per core (j in its 64-column slice):
  mask[i,j]   = max_b adj[b,i,j]                    (adj is 0/1)
  mh[b,j,f]   = sum_i mask[i,j] h[b,i,f]  ;  deg[j] = sum_i mask[i,j]
  me[b,j,e]   = sum_i mask[i,j] edge[b,i,j,e]
  msg[b,j,o]  = sum_f Wh[o,f] mh[b,j,f] + deg[j] wb[o] + sum_e We[o,e] me[b,j,e]
  out[b,j,o]  = sum_f U[o,f] (h[b,j,f] + msg[b,j,f]) + ub[o]
"""

import numpy as np


def _ensure_path():
    try:
        import concourse.bass  # noqa: F401
    except ImportError:
        import sys

        for p in ("/opt/trn_rl_repo", "/root/.axon_site/_ro/trn_rl_repo"):
            if p not in sys.path:
                sys.path.insert(0, p)


B, N, H, E = 8, 512, 64, 8
NCORES = 8
JB = N // NCORES  # 64 destination columns per core
CH = N // 128  # 4 source-node chunks of 128 partitions


_CACHE = {}


def _build_program():
    _ensure_path()
    import concourse.bacc as bacc
    import concourse.mybir as mybir
    import concourse.tile as tile

    dt = mybir.dt
    f32 = dt.float32
    i32 = dt.int32
    Alu = mybir.AluOpType

    nc = bacc.Bacc("TRN2", debug=False, num_devices=NCORES)

    edge = nc.dram_tensor("edge", [B, N, JB, E], f32, kind="ExternalInput").ap()
    adjs = nc.dram_tensor("adjs", [B, N, JB], i32, kind="ExternalInput").ap()
    h = nc.dram_tensor("h", [B, N, H], f32, kind="ExternalInput").ap()
    hs = nc.dram_tensor("hs", [B, JB, H], f32, kind="ExternalInput").ap()
    Ww = nc.dram_tensor("Ww", [H, H + E], f32, kind="ExternalInput").ap()
    Wb = nc.dram_tensor("Wb", [1, H], f32, kind="ExternalInput").ap()
    Uw = nc.dram_tensor("Uw", [H, H], f32, kind="ExternalInput").ap()
    Ub = nc.dram_tensor("Ub", [1, H], f32, kind="ExternalInput").ap()
    out = nc.dram_tensor("out", [B, JB, H], f32, kind="ExternalOutput").ap()

    ident_d = nc.inline_tensor(np.eye(128, dtype=np.float32), "ident")
    # DRAM bounce for the (j,e) -> [e, j] partition remap of the reduced
    # edge messages (PSUM cannot be DMA'd and engines cannot cross partitions).
    me_bounce = nc.dram_tensor("me_bounce", [B, JB * E], f32).ap()

    with tile.TileContext(nc) as tc:
        with (
            tc.tile_pool(name="const", bufs=1) as cpool,
            tc.tile_pool(name="edge", bufs=3) as epool,
            tc.tile_pool(name="masked", bufs=2) as mpool,
            tc.tile_pool(name="small", bufs=2) as spool,
            tc.tile_pool(name="pe", bufs=2, space="PSUM") as ppool_e,
            tc.tile_pool(name="pmh", bufs=2, space="PSUM") as ppool_mh,
            tc.tile_pool(name="pt", bufs=3, space="PSUM") as ppool_t,
            tc.tile_pool(name="pmsg", bufs=1, space="PSUM") as ppool_msg,
        ):
            # ---------------- constants ----------------
            ident = cpool.tile([128, 128], f32)
            nc.sync.dma_start(out=ident[:, :], in_=ident_d.ap()[:, :])
            ones_stat = cpool.tile([128, 1], f32)
            nc.gpsimd.memset(ones_stat[:, :], 1.0)

            Ww_sb = cpool.tile([H, H + E], f32)
            nc.sync.dma_start(out=Ww_sb[:, :], in_=Ww[:, :])
            Uw_sb = cpool.tile([H, H], f32)
            nc.sync.dma_start(out=Uw_sb[:, :], in_=Uw[:, :])

            # WhM = [Wh^T ; wb] (65 x 64), WeM = We^T (8 x 64), UM = [U^T ; ub]
            WhM = cpool.tile([H + 1, H], f32)
            WeM = cpool.tile([E, H], f32)
            UM = cpool.tile([H + 1, H], f32)

            pwh = ppool_t.tile([H, H], f32, tag="t", name="pwh")
            nc.tensor.transpose(pwh[:, :], Ww_sb[:, 0:H], ident[0:H, 0:H])
            nc.vector.tensor_copy(WhM[0:H, :], pwh[:, :])
            nc.sync.dma_start(out=WhM[H : H + 1, :], in_=Wb[:, :])

            pwe = ppool_t.tile([E, H], f32, tag="t", name="pwe")
            nc.tensor.transpose(pwe[:, :], Ww_sb[:, H : H + E], ident[0:H, 0:H])
            nc.vector.tensor_copy(WeM[:, :], pwe[:, :])

            puw = ppool_t.tile([H, H], f32, tag="t", name="puw")
            nc.tensor.transpose(puw[:, :], Uw_sb[:, :], ident[0:H, 0:H])
            nc.vector.tensor_copy(UM[0:H, :], puw[:, :])
            nc.sync.dma_start(out=UM[H : H + 1, :], in_=Ub[:, :])

            # ---------------- mask ----------------
            adj_sb = cpool.tile([128, B * CH * JB], i32)
            adj_v = adj_sb.rearrange("p (b c j) -> p b c j", b=B, c=CH)
            for b in range(B):
                nc.sync.dma_start(
                    out=adj_v[:, b],
                    in_=adjs[b].rearrange("(c p) j -> p c j", p=128),
                )
            mask = cpool.tile([128, CH * JB], f32)
            # reduce over the batch axis (innermost in the view) with max:
            # adj is 0/1 so max == (sum > 0)
            nc.vector.tensor_reduce(
                out=mask.rearrange("p (c j) -> p c j", c=CH),
                in_=adj_sb.rearrange("p (b c j) -> p c j b", b=B, c=CH),
                axis=mybir.AxisListType.X,
                op=Alu.max,
            )

            # broadcast view of the mask over the e axis (stride-0)
            mask_bcast = None
            try:
                mask_bcast = mask.rearrange("p (c j) -> p c j", c=CH).broadcast_to(
                    [128, CH, JB, E]
                )
            except Exception:
                mask_bcast = None
            if mask_bcast is None:
                mb = cpool.tile([128, CH * JB * E], f32)
                mb_v = mb.rearrange("p (c j e) -> p c j e", c=CH, j=JB)
                for e in range(E):
                    nc.vector.tensor_copy(
                        mb_v[:, :, :, e], mask.rearrange("p (c j) -> p c j", c=CH)
                    )
                mask_bcast = mb_v

            # ---------------- per-batch pipeline ----------------
            for b in range(B):
                edge_t = epool.tile([128, CH * JB * E], f32, name="edge_t")
                nc.sync.dma_start(
                    out=edge_t.rearrange("p (c j e) -> p c j e", c=CH, j=JB),
                    in_=edge[b].rearrange("(c p) j e -> p c j e", p=128),
                )

                masked = mpool.tile([128, CH * JB * E], f32, name="masked")
                nc.vector.tensor_tensor(
                    out=masked.rearrange("p (c j e) -> p c j e", c=CH, j=JB),
                    in0=edge_t.rearrange("p (c j e) -> p c j e", c=CH, j=JB),
                    in1=mask_bcast,
                    op=Alu.mult,
                )

                # reduce over i: psum_e[0, (j e)] = sum_i masked[i, (j e)]
                psum_e = ppool_e.tile([1, JB * E], f32, name="psum_e")
                for c in range(CH):
                    nc.tensor.matmul(
                        psum_e[:, :],
                        lhsT=ones_stat[:, :],
                        rhs=masked[:, c * JB * E : (c + 1) * JB * E],
                        start=(c == 0),
                        stop=(c == CH - 1),
                    )
                me_sb = spool.tile([1, JB * E], f32, name="me_sb")
                nc.scalar.copy(me_sb[:, :], psum_e[:, :])
                # remap (j,e) -> [e, j] partitions through a DRAM bounce
                nc.scalar.dma_start(out=me_bounce[b : b + 1], in_=me_sb[0:1, :])
                me_T = spool.tile([E, JB], f32, name="me_T")
                nc.scalar.dma_start(
                    out=me_T[:, :],
                    in_=me_bounce[b].rearrange("(j e) -> e j", e=E),
                )

                # mh | deg via mask^T @ [h | 1]
                h_plus = spool.tile([128, CH * (H + 1)], f32, name="h_plus")
                hp_v = h_plus.rearrange("p (c g) -> p c g", g=H + 1)
                nc.sync.dma_start(
                    out=hp_v[:, :, 0:H],
                    in_=h[b].rearrange("(c p) f -> p c f", p=128),
                )
                nc.gpsimd.memset(hp_v[:, :, H : H + 1], 1.0)

                psum_mh = ppool_mh.tile([JB, H + 1], f32, name="psum_mh")
                for c in range(CH):
                    nc.tensor.matmul(
                        psum_mh[:, :],
                        lhsT=mask[:, c * JB : (c + 1) * JB],
                        rhs=h_plus[:, c * (H + 1) : (c + 1) * (H + 1)],
                        start=(c == 0),
                        stop=(c == CH - 1),
                    )
                mh_sb = spool.tile([JB, H + 1], f32, name="mh_sb")
                nc.scalar.copy(mh_sb[:, :], psum_mh[:, :])

                psum_mhT = ppool_t.tile([H + 1, JB], f32, tag="t", name="psum_mhT")
                nc.tensor.transpose(psum_mhT[:, :], mh_sb[:, :], ident[0:JB, 0:JB])
                mhT_s = spool.tile([H + 1, JB], f32, name="mhT_s")
                nc.scalar.copy(mhT_s[:, :], psum_mhT[:, :])

                # messages (node part + bias + edge part) in one psum
                psum_msg = ppool_msg.tile([JB, H], f32, name="psum_msg")
                nc.tensor.matmul(
                    psum_msg[:, :], lhsT=mhT_s[:, :], rhs=WhM[:, :],
                    start=True, stop=False,
                )
                nc.tensor.matmul(
                    psum_msg[:, :], lhsT=me_T[:, :], rhs=WeM[:, :],
                    start=False, stop=True,
                )

                hs_t = spool.tile([JB, H], f32, name="hs_t")
                nc.sync.dma_start(out=hs_t[:, :], in_=hs[b])
                X = spool.tile([JB, H], f32, name="X")
                nc.vector.tensor_tensor(
                    out=X[:, :], in0=psum_msg[:, :], in1=hs_t[:, :], op=Alu.add
                )

                psum_xT = ppool_t.tile([H, JB], f32, tag="t", name="psum_xT")
                nc.tensor.transpose(psum_xT[:, :], X[:, :], ident[0:JB, 0:JB])
                XT_plus = spool.tile([H + 1, JB], f32, name="XT_plus")
                nc.vector.tensor_copy(XT_plus[0:H, :], psum_xT[:, :])
                nc.gpsimd.memset(XT_plus[H : H + 1, :], 1.0)

                psum_out = ppool_t.tile([JB, H], f32, tag="t", name="psum_out")
                nc.tensor.matmul(
                    psum_out[:, :], lhsT=XT_plus[:, :], rhs=UM[:, :],
                    start=True, stop=True,
                )
                out_sb = spool.tile([JB, H], f32, name="out_sb")
                nc.scalar.copy(out_sb[:, :], psum_out[:, :])
                nc.sync.dma_start(out=out[b], in_=out_sb[:, :])

    nc.compile()
    return nc


def _get_program():
    if "nc" not in _CACHE:
        _CACHE["nc"] = _build_program()
    return _CACHE["nc"]


def _make_in_maps(h, edge_attr, adj, W_w, W_b, U_w, U_b):
    h = np.ascontiguousarray(np.asarray(h, dtype=np.float32))
    edge_attr = np.asarray(edge_attr, dtype=np.float32)
    adj = np.asarray(adj, dtype=np.int32)
    W_w = np.ascontiguousarray(np.asarray(W_w, dtype=np.float32))
    W_b = np.ascontiguousarray(np.asarray(W_b, dtype=np.float32)).reshape(1, H)
    U_w = np.ascontiguousarray(np.asarray(U_w, dtype=np.float32))
    U_b = np.ascontiguousarray(np.asarray(U_b, dtype=np.float32)).reshape(1, H)

    in_maps = []
    for c in range(NCORES):
        j0 = c * JB
        in_maps.append(
            {
                "edge": np.ascontiguousarray(edge_attr[:, :, j0 : j0 + JB, :]),
                "adjs": np.ascontiguousarray(adj[:, :, j0 : j0 + JB]),
                "h": h,
                "hs": np.ascontiguousarray(h[:, j0 : j0 + JB, :]),
                "Ww": W_w,
                "Wb": W_b,
                "Uw": U_w,
                "Ub": U_b,
            }
        )
    return in_maps


def _install_ntff_hook():
    """The agent image lacks antenv.axon_hooks; synthesize it so trace=True
    can reach the libaxon NTFF profiling entry points."""
    import sys
    import types

    try:
        from antenv.axon_hooks import get_axon_ntff_profile_hook  # noqa: F401

        return
    except ImportError:
        pass
    import antenv

    mod = types.ModuleType("antenv.axon_hooks")
    _h = [None]
    mod.set_axon_ntff_profile_hook = lambda hook: _h.__setitem__(0, hook)
    mod.get_axon_ntff_profile_hook = lambda: _h[0]
    sys.modules["antenv.axon_hooks"] = mod
    antenv.axon_hooks = mod
    try:
        from trn_agent_boot.trn_boot import _ntff_profile_via_ctypes

        mod.set_axon_ntff_profile_hook(
            _ntff_profile_via_ctypes("/opt/axon/libaxon_pjrt.so")
        )
    except Exception:
        pass
    # avoid the bucket upload (no bucket in this container)
    import concourse.bass_utils as bu

    bu.upload_artifacts = lambda tmpdir: str(tmpdir)


def run(h, edge_attr, adj, W_w, W_b, U_w, U_b, trace=False, trace_cores=None):
    """Run the kernel; returns (output, BassKernelResults)."""
    _ensure_path()
    if trace:
        _install_ntff_hook()
    from concourse.bass_utils import run_bass_kernel_spmd

    nc = _get_program()
    in_maps = _make_in_maps(h, edge_attr, adj, W_w, W_b, U_w, U_b)
    kw = {}
    if trace:
        kw = {"trace": True, "trace_cores": trace_cores or [0]}
    res = run_bass_kernel_spmd(nc, in_maps, list(range(NCORES)), **kw)
    outs = [res.results[c]["out"] for c in range(NCORES)]
    full = np.concatenate(outs, axis=1)  # [B, N, H]
    return full, res


def kernel(h, edge_attr, adj, W_w, W_b, U_w, U_b):
    full, _ = run(h, edge_attr, adj, W_w, W_b, U_w, U_b)
    return full


# revision 14
# speedup vs baseline: 1.0139x; 1.0139x over previous
"""DMPNN layer kernel for 8 Trainium2 NeuronCores.

Sharding: data-parallel over destination nodes j (dim 2 of edge_attr/adj,
dim 1 of the output). Each core gets a 64-column j-slice of edge_attr/adj,
the full h (needed because messages sum over all source nodes i), and the
small weights replicated. The batch-global mask (adj.sum(0) > 0) only needs
the core's own j-slice of adj over the full batch, so no collective at all.

Math per core (j in its 64-column slice):
  mask[i,j]   = max_b adj[b,i,j]                    (adj is 0/1)
  mh[b,j,f]   = sum_i mask[i,j] h[b,i,f]  ;  deg[j] = sum_i mask[i,j]
  me[b,j,e]   = sum_i mask[i,j] edge[b,i,j,e]
  msg[b,j,o]  = sum_f Wh[o,f] mh[b,j,f] + deg[j] wb[o] + sum_e We[o,e] me[b,j,e]
  out[b,j,o]  = sum_f U[o,f] (h[b,j,f] + msg[b,j,f]) + ub[o]
"""

import numpy as np


def _ensure_path():
    try:
        import concourse.bass  # noqa: F401
    except ImportError:
        import sys

        for p in ("/opt/trn_rl_repo", "/root/.axon_site/_ro/trn_rl_repo"):
            if p not in sys.path:
                sys.path.insert(0, p)


B, N, H, E = 8, 512, 64, 8
NCORES = 8
JB = N // NCORES  # 64 destination columns per core
CH = N // 128  # 4 source-node chunks of 128 partitions


_CACHE = {}


def _build_program():
    _ensure_path()
    import concourse.bacc as bacc
    import concourse.mybir as mybir
    import concourse.tile as tile

    dt = mybir.dt
    f32 = dt.float32
    i32 = dt.int32
    Alu = mybir.AluOpType

    nc = bacc.Bacc("TRN2", debug=False, num_devices=NCORES)

    edge = nc.dram_tensor("edge", [B, N, JB, E], f32, kind="ExternalInput").ap()
    adjs = nc.dram_tensor("adjs", [B, N, JB], i32, kind="ExternalInput").ap()
    h = nc.dram_tensor("h", [B, N, H], f32, kind="ExternalInput").ap()
    hs = nc.dram_tensor("hs", [B, JB, H], f32, kind="ExternalInput").ap()
    Ww = nc.dram_tensor("Ww", [H, H + E], f32, kind="ExternalInput").ap()
    Wb = nc.dram_tensor("Wb", [1, H], f32, kind="ExternalInput").ap()
    Uw = nc.dram_tensor("Uw", [H, H], f32, kind="ExternalInput").ap()
    Ub = nc.dram_tensor("Ub", [1, H], f32, kind="ExternalInput").ap()
    out = nc.dram_tensor("out", [B, JB, H], f32, kind="ExternalOutput").ap()

    ident_d = nc.inline_tensor(np.eye(128, dtype=np.float32), "ident")
    # DRAM bounce for the (j,e) -> [e, j] partition remap of the reduced
    # edge messages (PSUM cannot be DMA'd and engines cannot cross partitions).
    me_bounce = nc.dram_tensor("me_bounce", [B, JB * E], f32).ap()

    with tile.TileContext(nc) as tc:
        with (
            tc.tile_pool(name="const", bufs=1) as cpool,
            tc.tile_pool(name="edge", bufs=4) as epool,
            tc.tile_pool(name="masked", bufs=3) as mpool,
            tc.tile_pool(name="small", bufs=3) as spool,
            tc.tile_pool(name="pe", bufs=2, space="PSUM") as ppool_e,
            tc.tile_pool(name="pmh", bufs=2, space="PSUM") as ppool_mh,
            tc.tile_pool(name="pt", bufs=3, space="PSUM") as ppool_t,
            tc.tile_pool(name="pmsg", bufs=1, space="PSUM") as ppool_msg,
        ):
            # ---------------- mask first: it gates the whole pipeline -----
            adj_sb = cpool.tile([128, B * CH * JB], i32)
            adj_v = adj_sb.rearrange("p (b c j) -> p b c j", b=B, c=CH)
            for b in range(B):
                nc.sync.dma_start(
                    out=adj_v[:, b],
                    in_=adjs[b].rearrange("(c p) j -> p c j", p=128),
                )
            mask = cpool.tile([128, CH * JB], f32)
            # reduce over the batch axis (innermost in the view) with max:
            # adj is 0/1 so max == (sum > 0)
            nc.vector.tensor_reduce(
                out=mask.rearrange("p (c j) -> p c j", c=CH),
                in_=adj_sb.rearrange("p (b c j) -> p c j b", b=B, c=CH),
                axis=mybir.AxisListType.X,
                op=Alu.max,
            )

            # ---------------- constants ----------------
            ident = cpool.tile([128, 128], f32)
            nc.scalar.dma_start(out=ident[:, :], in_=ident_d.ap()[:, :])
            ones_stat = cpool.tile([128, 1], f32)
            nc.gpsimd.memset(ones_stat[:, :], 1.0)

            Ww_sb = cpool.tile([H, H + E], f32)
            nc.scalar.dma_start(out=Ww_sb[:, :], in_=Ww[:, :])
            Uw_sb = cpool.tile([H, H], f32)
            nc.scalar.dma_start(out=Uw_sb[:, :], in_=Uw[:, :])

            # WhM = [Wh^T ; wb] (65 x 64), WeM = We^T (8 x 64), UM = [U^T ; ub]
            WhM = cpool.tile([H + 1, H], f32)
            WeM = cpool.tile([E, H], f32)
            UM = cpool.tile([H + 1, H], f32)

            pwh = ppool_t.tile([H, H], f32, tag="t", name="pwh")
            nc.tensor.transpose(pwh[:, :], Ww_sb[:, 0:H], ident[0:H, 0:H])
            nc.vector.tensor_copy(WhM[0:H, :], pwh[:, :])
            nc.scalar.dma_start(out=WhM[H : H + 1, :], in_=Wb[:, :])

            pwe = ppool_t.tile([E, H], f32, tag="t", name="pwe")
            nc.tensor.transpose(pwe[:, :], Ww_sb[:, H : H + E], ident[0:H, 0:H])
            nc.vector.tensor_copy(WeM[:, :], pwe[:, :])

            puw = ppool_t.tile([H, H], f32, tag="t", name="puw")
            nc.tensor.transpose(puw[:, :], Uw_sb[:, :], ident[0:H, 0:H])
            nc.vector.tensor_copy(UM[0:H, :], puw[:, :])
            nc.scalar.dma_start(out=UM[H : H + 1, :], in_=Ub[:, :])

            # broadcast view of the mask over the e axis (stride-0)
            mask_bcast = None
            try:
                mask_bcast = mask.rearrange("p (c j) -> p c j", c=CH).broadcast_to(
                    [128, CH, JB, E]
                )
            except Exception:
                mask_bcast = None
            if mask_bcast is None:
                mb = cpool.tile([128, CH * JB * E], f32)
                mb_v = mb.rearrange("p (c j e) -> p c j e", c=CH, j=JB)
                for e in range(E):
                    nc.vector.tensor_copy(
                        mb_v[:, :, :, e], mask.rearrange("p (c j) -> p c j", c=CH)
                    )
                mask_bcast = mb_v

            # ---------------- per-batch pipeline ----------------
            for b in range(B):
                edge_t = epool.tile([128, CH * JB * E], f32, name="edge_t")
                nc.sync.dma_start(
                    out=edge_t.rearrange("p (c j e) -> p c j e", c=CH, j=JB),
                    in_=edge[b].rearrange("(c p) j e -> p c j e", p=128),
                )

                # mask-multiply per i-chunk so each ones-matmul can start as
                # soon as its 512-column slab is masked
                masked = mpool.tile([128, CH * JB * E], f32, name="masked")
                mk_v = masked.rearrange("p (c j e) -> p c j e", c=CH, j=JB)
                eg_v = edge_t.rearrange("p (c j e) -> p c j e", c=CH, j=JB)
                psum_e = ppool_e.tile([1, JB * E], f32, name="psum_e")
                for c in range(CH):
                    nc.vector.tensor_tensor(
                        out=mk_v[:, c],
                        in0=eg_v[:, c],
                        in1=mask_bcast[:, c],
                        op=Alu.mult,
                    )
                    nc.tensor.matmul(
                        psum_e[:, :],
                        lhsT=ones_stat[:, :],
                        rhs=masked[:, c * JB * E : (c + 1) * JB * E],
                        start=(c == 0),
                        stop=(c == CH - 1),
                    )
                me_sb = spool.tile([1, JB * E], f32, name="me_sb")
                nc.scalar.copy(me_sb[:, :], psum_e[:, :])
                # remap (j,e) -> [e, j] partitions through a DRAM bounce
                nc.scalar.dma_start(out=me_bounce[b : b + 1], in_=me_sb[0:1, :])
                me_T = spool.tile([E, JB], f32, name="me_T")
                nc.scalar.dma_start(
                    out=me_T[:, :],
                    in_=me_bounce[b].rearrange("(j e) -> e j", e=E),
                )

                # mh | deg via mask^T @ [h | 1]
                h_plus = spool.tile([128, CH * (H + 1)], f32, name="h_plus")
                hp_v = h_plus.rearrange("p (c g) -> p c g", g=H + 1)
                nc.scalar.dma_start(
                    out=hp_v[:, :, 0:H],
                    in_=h[b].rearrange("(c p) f -> p c f", p=128),
                )
                nc.gpsimd.memset(hp_v[:, :, H : H + 1], 1.0)

                psum_mh = ppool_mh.tile([JB, H + 1], f32, name="psum_mh")
                for c in range(CH):
                    nc.tensor.matmul(
                        psum_mh[:, :],
                        lhsT=mask[:, c * JB : (c + 1) * JB],
                        rhs=h_plus[:, c * (H + 1) : (c + 1) * (H + 1)],
                        start=(c == 0),
                        stop=(c == CH - 1),
                    )
                mh_sb = spool.tile([JB, H + 1], f32, name="mh_sb")
                nc.scalar.copy(mh_sb[:, :], psum_mh[:, :])

                psum_mhT = ppool_t.tile([H + 1, JB], f32, tag="t", name="psum_mhT")
                nc.tensor.transpose(psum_mhT[:, :], mh_sb[:, :], ident[0:JB, 0:JB])
                mhT_s = spool.tile([H + 1, JB], f32, name="mhT_s")
                nc.scalar.copy(mhT_s[:, :], psum_mhT[:, :])

                # messages (node part + bias + edge part) in one psum
                psum_msg = ppool_msg.tile([JB, H], f32, name="psum_msg")
                nc.tensor.matmul(
                    psum_msg[:, :], lhsT=mhT_s[:, :], rhs=WhM[:, :],
                    start=True, stop=False,
                )
                nc.tensor.matmul(
                    psum_msg[:, :], lhsT=me_T[:, :], rhs=WeM[:, :],
                    start=False, stop=True,
                )

                hs_t = spool.tile([JB, H], f32, name="hs_t")
                nc.scalar.dma_start(out=hs_t[:, :], in_=hs[b])
                X = spool.tile([JB, H], f32, name="X")
                nc.vector.tensor_tensor(
                    out=X[:, :], in0=psum_msg[:, :], in1=hs_t[:, :], op=Alu.add
                )

                psum_xT = ppool_t.tile([H, JB], f32, tag="t", name="psum_xT")
                nc.tensor.transpose(psum_xT[:, :], X[:, :], ident[0:JB, 0:JB])
                XT_plus = spool.tile([H + 1, JB], f32, name="XT_plus")
                nc.vector.tensor_copy(XT_plus[0:H, :], psum_xT[:, :])
                nc.gpsimd.memset(XT_plus[H : H + 1, :], 1.0)

                psum_out = ppool_t.tile([JB, H], f32, tag="t", name="psum_out")
                nc.tensor.matmul(
                    psum_out[:, :], lhsT=XT_plus[:, :], rhs=UM[:, :],
                    start=True, stop=True,
                )
                out_sb = spool.tile([JB, H], f32, name="out_sb")
                nc.scalar.copy(out_sb[:, :], psum_out[:, :])
                nc.scalar.dma_start(out=out[b], in_=out_sb[:, :])

    nc.compile()
    return nc


def _get_program():
    if "nc" not in _CACHE:
        _CACHE["nc"] = _build_program()
    return _CACHE["nc"]


def _make_in_maps(h, edge_attr, adj, W_w, W_b, U_w, U_b):
    h = np.ascontiguousarray(np.asarray(h, dtype=np.float32))
    edge_attr = np.asarray(edge_attr, dtype=np.float32)
    adj = np.asarray(adj, dtype=np.int32)
    W_w = np.ascontiguousarray(np.asarray(W_w, dtype=np.float32))
    W_b = np.ascontiguousarray(np.asarray(W_b, dtype=np.float32)).reshape(1, H)
    U_w = np.ascontiguousarray(np.asarray(U_w, dtype=np.float32))
    U_b = np.ascontiguousarray(np.asarray(U_b, dtype=np.float32)).reshape(1, H)

    in_maps = []
    for c in range(NCORES):
        j0 = c * JB
        in_maps.append(
            {
                "edge": np.ascontiguousarray(edge_attr[:, :, j0 : j0 + JB, :]),
                "adjs": np.ascontiguousarray(adj[:, :, j0 : j0 + JB]),
                "h": h,
                "hs": np.ascontiguousarray(h[:, j0 : j0 + JB, :]),
                "Ww": W_w,
                "Wb": W_b,
                "Uw": U_w,
                "Ub": U_b,
            }
        )
    return in_maps


def _install_ntff_hook():
    """The agent image lacks antenv.axon_hooks; synthesize it so trace=True
    can reach the libaxon NTFF profiling entry points."""
    import sys
    import types

    try:
        from antenv.axon_hooks import get_axon_ntff_profile_hook  # noqa: F401

        return
    except ImportError:
        pass
    import antenv

    mod = types.ModuleType("antenv.axon_hooks")
    _h = [None]
    mod.set_axon_ntff_profile_hook = lambda hook: _h.__setitem__(0, hook)
    mod.get_axon_ntff_profile_hook = lambda: _h[0]
    sys.modules["antenv.axon_hooks"] = mod
    antenv.axon_hooks = mod
    try:
        from trn_agent_boot.trn_boot import _ntff_profile_via_ctypes

        mod.set_axon_ntff_profile_hook(
            _ntff_profile_via_ctypes("/opt/axon/libaxon_pjrt.so")
        )
    except Exception:
        pass
    # avoid the bucket upload (no bucket in this container)
    import concourse.bass_utils as bu

    bu.upload_artifacts = lambda tmpdir: str(tmpdir)


def run(h, edge_attr, adj, W_w, W_b, U_w, U_b, trace=False, trace_cores=None):
    """Run the kernel; returns (output, BassKernelResults)."""
    _ensure_path()
    if trace:
        _install_ntff_hook()
    from concourse.bass_utils import run_bass_kernel_spmd

    nc = _get_program()
    in_maps = _make_in_maps(h, edge_attr, adj, W_w, W_b, U_w, U_b)
    kw = {}
    if trace:
        kw = {"trace": True, "trace_cores": trace_cores or [0]}
    res = run_bass_kernel_spmd(nc, in_maps, list(range(NCORES)), **kw)
    outs = [res.results[c]["out"] for c in range(NCORES)]
    full = np.concatenate(outs, axis=1)  # [B, N, H]
    return full, res


def kernel(h, edge_attr, adj, W_w, W_b, U_w, U_b):
    full, _ = run(h, edge_attr, adj, W_w, W_b, U_w, U_b)
    return full
